# revision 15
# baseline (speedup 1.0000x reference)
"""DGMG loss kernel for Trainium2, 8-core data-parallel over graphs.

Contract: kernel(**inputs) takes the FULL unsharded inputs (as in
reference.setup_inputs()) and returns the FULL output (scalar f32 loss).

Strategy:
- B=256 graphs of N=128 nodes -> 32 graphs per core. N == 128 partitions.
- Per-core state lives entirely in SBUF, feature-major: hVT[p, f, g, s].
- segment_sum over the (fixed) edge list becomes a per-graph dense
  128x128 adjacency matmul; the adjacency is built on host from src/dst.
- readout = colsum(hV_g) @ gpW + N*gpb  (segment_sum of a linear map).
- All PE matmuls run in bf16 (fp32 PSUM accumulation); validated to give
  ~3e-4 relative error on the final loss.
- The scatter-step readout is updated incrementally: hG += (hv-old)@gpW.
- log_softmax over the 2-class fan head folds to softplus(l1-l0).
- The per-graph constant part of the fs MLP enters the big matmul as an
  extra K=32 rank with a fixed 0/1 indicator rhs.
"""
import sys
from contextlib import ExitStack

sys.path.insert(0, "/opt/trn_rl_repo")

import numpy as np
import ml_dtypes

import concourse.bacc as bacc
import concourse.tile as tile
import concourse.mybir as mybir
from concourse import bass_utils
from concourse.masks import make_identity

BF = mybir.dt.bfloat16
F32 = mybir.dt.float32
AF = mybir.ActivationFunctionType
ALU = mybir.AluOpType
AX = mybir.AxisListType

B, N, D, G = 256, 128, 256, 512
S, T = 4, 2
NCORES = 8
GBL = B // NCORES          # 32 graphs per core
NF = D // 128              # 2 feature tiles
NG = G // 128              # 4 graph-hidden tiles
NCH = GBL * N // 512       # 8 chunks of 512 over (g, s)
EPS = 1e-7

_BUILT = None  # cached (nc, meta)


# --------------------------------------------------------------------------
# device kernel builder
# --------------------------------------------------------------------------

def _declare_inputs(nc):
    d = {}

    def di(name, shape, dt):
        d[name] = nc.dram_tensor(name, list(shape), dt, kind="ExternalInput")

    di("hVT0", (128, NF * GBL * N), BF)
    di("AT", (128, GBL * N), BF)
    di("wfan1", (128, 4 * 4 * 128), BF)
    di("bfan1", (128, 4), F32)
    di("wfan2d", (128, 4), BF)
    di("wfinit1", (128, 4 * 4 * 128), BF)
    di("bfinit1", (128, 4), F32)
    di("wfinit2", (128, 4 * 2 * 128), BF)
    di("bfinit2", (128, 2), F32)
    di("wfae1", (128, 6 * 6 * 128), BF)
    di("bfae1", (128, 6), F32)
    di("wfae2", (128, 6), BF)
    di("wfs1a", (128, 2 * 4 * 128), BF)
    di("wfs1b", (128, 2 * 4 * 128), BF)
    di("bfs1", (128, 4), F32)
    di("wfs2", (128, 4), BF)
    di("wgp", (128, 2 * 4 * 128), BF)
    di("bgpN", (128, 4), F32)
    di("wgcn", (128, T * 2 * 2 * 128), BF)
    di("bgcn", (128, T * 2), F32)
    di("ind", (32, GBL * N), BF)
    di("labn", (1, S * GBL), F32)
    di("labe", (1, S * GBL), F32)
    di("selhot", (32, S * N), F32)
    di("consts", (1, 4), F32)  # [fan_b2d, fae_b2, eps, 1+eps]
    return d


def _build():
    import os as _os
    PH = int(_os.environ.get("DGMG_PHASES", "9"))
    nc = bacc.Bacc("TRN2", target_bir_lowering=False, debug=False)
    dins = _declare_inputs(nc)
    dout = nc.dram_tensor("lossout", [1, 1], F32, kind="ExternalOutput")

    with tile.TileContext(nc) as tc, ExitStack() as stk:
        cp = stk.enter_context(tc.tile_pool(name="const", bufs=1))
        wp = stk.enter_context(tc.tile_pool(name="work", bufs=2))
        pp = stk.enter_context(tc.tile_pool(name="ps", bufs=2, space="PSUM"))

        # ---- persistent SBUF state ----
        hVT = cp.tile([128, NF, GBL, N], BF)            # node hidden, feature-major
        hGT = cp.tile([128, NG, GBL], F32)              # graph hidden, feature-major
        AT = cp.tile([128, GBL, N], BF)
        wfan1 = cp.tile([128, 4, 4, 128], BF)
        bfan1 = cp.tile([128, 4], F32)
        wfan2d = cp.tile([128, 4], BF)
        wfinit1 = cp.tile([128, 4, 4, 128], BF)
        bfinit1 = cp.tile([128, 4], F32)
        wfinit2 = cp.tile([128, 4, 2, 128], BF)
        bfinit2 = cp.tile([128, 2], F32)
        wfae1 = cp.tile([128, 6, 6, 128], BF)
        bfae1 = cp.tile([128, 6], F32)
        wfae2 = cp.tile([128, 6], BF)
        wfs1a = cp.tile([128, 2, 4, 128], BF)
        wfs1b = cp.tile([128, 2, 4, 128], BF)
        bfs1 = cp.tile([128, 4], F32)
        wfs2 = cp.tile([128, 4], BF)
        wgp = cp.tile([128, 2, 4, 128], BF)
        bgpN = cp.tile([128, 4], F32)
        wgcn = cp.tile([128, T, 2, 2, 128], BF)
        bgcn = cp.tile([128, T, 2], F32)
        ind = cp.tile([32, GBL * N], BF)
        labn = cp.tile([1, S * GBL], F32)
        labe = cp.tile([1, S * GBL], F32)
        selhot = cp.tile([32, S * N], F32)
        consts = cp.tile([1, 4], F32)
        identity = cp.tile([128, 128], BF)
        rowacc = cp.tile([1, GBL], F32)
        colacc = cp.tile([GBL, 1], F32)
        ones32 = cp.tile([GBL, 1], F32)
        hGT_bf = cp.tile([128, NG, GBL], BF)

        # ---- load everything ----
        for name, t in [
            ("hVT0", hVT), ("AT", AT), ("wfan1", wfan1), ("bfan1", bfan1),
            ("wfan2d", wfan2d), ("wfinit1", wfinit1), ("bfinit1", bfinit1),
            ("wfinit2", wfinit2), ("bfinit2", bfinit2), ("wfae1", wfae1),
            ("bfae1", bfae1), ("wfae2", wfae2), ("wfs1a", wfs1a),
            ("wfs1b", wfs1b), ("bfs1", bfs1), ("wfs2", wfs2), ("wgp", wgp),
            ("bgpN", bgpN), ("wgcn", wgcn), ("bgcn", bgcn), ("ind", ind),
            ("labn", labn), ("labe", labe), ("selhot", selhot),
            ("consts", consts),
        ]:
            ap = t[:]
            if len(ap.shape) > 2:
                spec = {3: "p a b -> p (a b)", 4: "p a b c -> p (a b c)",
                        5: "p a b c d -> p (a b c d)"}[len(ap.shape)]
                ap = ap.rearrange(spec)
            nc.sync.dma_start(out=ap, in_=dins[name].ap())

        make_identity(nc, identity[:])
        nc.vector.memset(rowacc[:], 0.0)
        nc.vector.memset(colacc[:], 0.0)
        nc.vector.memset(ones32[:], 1.0)

        # ---- helpers ----
        def readout_full():
            """hGT <- colsum(hVT) @ gpW + N*gpb  (overwrites hGT, refreshes hGT_bf)"""
            colsumT = wp.tile([128, NF, GBL], F32, name="colsumT")
            for f in range(NF):
                nc.vector.tensor_reduce(
                    out=colsumT[:, f, :], in_=hVT[:, f, :, :], axis=AX.X, op=ALU.add)
            colsum_bf = wp.tile([128, NF, GBL], BF, name="colsum_bf")
            nc.vector.tensor_copy(out=colsum_bf[:], in_=colsumT[:])
            hgps = pp.tile([128, NG, GBL], F32, name="hgps", tag="sp")
            for ko in range(NG):
                for ki in range(NF):
                    nc.tensor.matmul(
                        out=hgps[:, ko, :], lhsT=wgp[:, ki, ko, :],
                        rhs=colsum_bf[:, ki, :],
                        start=(ki == 0), stop=(ki == NF - 1))
                nc.scalar.activation(
                    out=hGT[:, ko, :], in_=hgps[:, ko, :], func=AF.Identity,
                    bias=bgpN[:, ko:ko + 1])
            nc.vector.tensor_copy(out=hGT_bf[:], in_=hGT[:])

        def mlp_to_psum(psum, win, bin_, rhs_tiles, nk, nko, act_out=None):
            """psum[:, ko, :] = sum_ki win[:,ki,ko,:] .T @ rhs_tiles(ki); then
            act_out[:, ko, :] = sigmoid(psum + bin_[:, ko])."""
            for ko in range(nko):
                for ki in range(nk):
                    nc.tensor.matmul(
                        out=psum[:, ko, :], lhsT=win[:, ki, ko, :],
                        rhs=rhs_tiles(ki), start=(ki == 0), stop=(ki == nk - 1))
                if act_out is not None:
                    nc.scalar.activation(
                        out=act_out[:, ko, :], in_=psum[:, ko, :],
                        func=AF.Sigmoid, bias=bin_[:, ko:ko + 1])

        # ---- initial readout ----
        readout_full()

        # ---- generation steps ----
        for s in range(S if PH >= 1 else 0):
            labn_s = labn[:, s * GBL:(s + 1) * GBL]
            labe_s = labe[:, s * GBL:(s + 1) * GBL]
            sel_s = selhot[:, s * N:(s + 1) * N]

            # ---------- fan: decide_add_node + loss1 ----------
            fanps = pp.tile([128, 4, GBL], F32, name="fanps", tag="sp")
            h1fan = wp.tile([128, 4, GBL], BF, name="h1fan")
            mlp_to_psum(fanps, wfan1, bfan1, lambda ki: hGT_bf[:, ki, :], 4, 4,
                        act_out=h1fan)
            dps = pp.tile([1, GBL], F32, name="dps", tag="sp")
            for k in range(4):
                nc.tensor.matmul(out=dps[:], lhsT=wfan2d[:, k:k + 1],
                                 rhs=h1fan[:, k, :], start=(k == 0), stop=(k == 3))
            sp = wp.tile([1, GBL], F32, name="sp")
            expd = wp.tile([1, GBL], F32, name="expd")
            draw = wp.tile([1, GBL], F32, name="draw")
            # softplus(d + b2d) = ln(1 + exp(d + b2d))
            nc.scalar.activation(out=expd[:], in_=dps[:], func=AF.Exp,
                                 bias=consts[:, 0:1])
            nc.scalar.activation(out=sp[:], in_=expd[:], func=AF.Ln, bias=1.0)
            nc.scalar.activation(out=draw[:], in_=dps[:], func=AF.Identity,
                                 bias=consts[:, 0:1])
            t1row = wp.tile([1, GBL], F32, name="t1row")
            nc.vector.tensor_add(out=rowacc[:], in0=rowacc[:], in1=sp[:])
            nc.vector.tensor_mul(out=t1row[:], in0=draw[:], in1=labn_s)
            nc.vector.tensor_sub(out=rowacc[:], in0=rowacc[:], in1=t1row[:])

            # ---------- finit -> hv ----------
            g1ps = pp.tile([128, 4, GBL], F32, name="g1ps", tag="sp")
            g1T = wp.tile([128, 4, GBL], BF, name="g1T")
            mlp_to_psum(g1ps, wfinit1, bfinit1, lambda ki: hGT_bf[:, ki, :], 4, 4,
                        act_out=g1T)
            hvps = pp.tile([128, NF, GBL], F32, name="hvps", tag="sp")
            hvT = wp.tile([128, NF, GBL], BF, name="hvT")
            for ko in range(NF):
                for ki in range(4):
                    nc.tensor.matmul(
                        out=hvps[:, ko, :], lhsT=wfinit2[:, ki, ko, :],
                        rhs=g1T[:, ki, :], start=(ki == 0), stop=(ki == 3))
                nc.scalar.activation(
                    out=hvT[:, ko, :], in_=hvps[:, ko, :], func=AF.Identity,
                    bias=bfinit2[:, ko:ko + 1])

            # ---------- scatter node s + incremental readout ----------
            diffbf = wp.tile([128, NF, GBL], BF, name="diffbf")
            nc.vector.tensor_sub(out=diffbf[:], in0=hvT[:], in1=hVT[:, :, :, s])
            nc.vector.tensor_copy(out=hVT[:, :, :, s], in_=hvT[:])
            dhg = pp.tile([128, NG, GBL], F32, name="dhg", tag="sp")
            for ko in range(NG):
                for ki in range(NF):
                    nc.tensor.matmul(
                        out=dhg[:, ko, :], lhsT=wgp[:, ki, ko, :],
                        rhs=diffbf[:, ki, :], start=(ki == 0), stop=(ki == NF - 1))
            nc.vector.tensor_add(out=hGT[:], in0=hGT[:], in1=dhg[:])
            nc.vector.tensor_copy(out=hGT_bf[:], in_=hGT[:])

            # ---------- fae: decide_add_edge + loss2 ----------
            if PH < 2:
                continue
            ups = pp.tile([128, 6, GBL], F32, name="ups", tag="sp")
            u1T = wp.tile([128, 6, GBL], BF, name="u1T")

            def fae_rhs(ki):
                return hGT_bf[:, ki, :] if ki < 4 else hvT[:, ki - 4, :]

            mlp_to_psum(ups, wfae1, bfae1, fae_rhs, 6, 6, act_out=u1T)
            peps = pp.tile([1, GBL], F32, name="peps", tag="sp")
            for k in range(6):
                nc.tensor.matmul(out=peps[:], lhsT=wfae2[:, k:k + 1],
                                 rhs=u1T[:, k, :], start=(k == 0), stop=(k == 5))
            pesb = wp.tile([1, GBL], F32, name="pesb")
            nc.scalar.activation(out=pesb[:], in_=peps[:], func=AF.Sigmoid,
                                 bias=consts[:, 1:2])
            t1e = wp.tile([1, GBL], F32, name="t1e")
            t2e = wp.tile([1, GBL], F32, name="t2e")
            nc.scalar.activation(out=t1e[:], in_=pesb[:], func=AF.Ln, bias=consts[:, 2:3])
            nc.scalar.activation(out=t2e[:], in_=pesb[:], func=AF.Ln,
                                 scale=-1.0, bias=consts[:, 3:4])
            d12 = wp.tile([1, GBL], F32, name="d12")
            nc.vector.tensor_sub(out=d12[:], in0=t1e[:], in1=t2e[:])
            nc.vector.tensor_sub(out=rowacc[:], in0=rowacc[:], in1=t2e[:])
            nc.vector.tensor_mul(out=d12[:], in0=d12[:], in1=labe_s)
            nc.vector.tensor_sub(out=rowacc[:], in0=rowacc[:], in1=d12[:])

            # ---------- fs: select_node_to_add_edge + loss3 ----------
            if PH < 3:
                continue
            FSUB = int(_os.environ.get("DGMG_FS_SUB", "9"))
            # cst[g, :] = fs_w1b^T hv_g + fs_b1, then transpose to [g, fo]
            cstps = pp.tile([128, 4, GBL], F32, name="cstps", tag="sp")
            cst_fm = wp.tile([128, 4, GBL], BF, name="cst_fm")
            for ko in range(4):
                for ki in range(NF):
                    nc.tensor.matmul(
                        out=cstps[:, ko, :], lhsT=wfs1b[:, ki, ko, :],
                        rhs=hvT[:, ki, :], start=(ki == 0), stop=(ki == NF - 1))
                nc.scalar.activation(
                    out=cst_fm[:, ko, :], in_=cstps[:, ko, :], func=AF.Identity,
                    bias=bfs1[:, ko:ko + 1])
            if FSUB < 2:
                continue
            cstT = wp.tile([32, 4, 128], BF, name="cstT")
            csttps = pp.tile([32, 4, 128], BF, name="csttps", tag="sp")
            for ko in range(4):
                nc.tensor.transpose(out=csttps[:, ko, :], in_=cst_fm[:, ko, :],
                                    identity=identity[:])
                nc.vector.tensor_copy(out=cstT[:, ko, :], in_=csttps[:, ko, :])

            if FSUB < 3:
                continue
            scrow = wp.tile([1, GBL * N], F32, name="scrow")
            hVTf = [hVT[:, f, :, :].rearrange("p g s -> p (g s)") for f in range(NF)]
            for ch in range(NCH):
                cols = slice(ch * 512, (ch + 1) * 512)
                h1c = wp.tile([128, 4, 512], BF, name="h1c")
                for ko in range(4):
                    zps = pp.tile([128, 512], F32, name="zps", tag="zp")
                    nc.tensor.matmul(out=zps[:], lhsT=cstT[:, ko, :],
                                     rhs=ind[:, cols], start=True, stop=False)
                    for ki in range(NF):
                        nc.tensor.matmul(
                            out=zps[:], lhsT=wfs1a[:, ki, ko, :],
                            rhs=hVTf[ki][:, cols],
                            start=False, stop=(ki == NF - 1))
                    nc.scalar.activation(out=h1c[:, ko, :], in_=zps[:],
                                         func=AF.Sigmoid)
                scps = pp.tile([1, 512], F32, name="scps", tag="sp")
                for ko in range(4):
                    nc.tensor.matmul(out=scps[:], lhsT=wfs2[:, ko:ko + 1],
                                     rhs=h1c[:, ko, :], start=(ko == 0),
                                     stop=(ko == 3))
                nc.scalar.copy(out=scrow[:, cols], in_=scps[:])

            if FSUB < 4:
                continue
            s32 = wp.tile([32, N], F32, name="s32")
            import os as _os
            if _os.environ.get("DGMG_NO_SCOREDMA"):
                nc.vector.memset(s32[:], 0.01)
            elif _os.environ.get("DGMG_ROWDMA"):
                for gq in range(8):
                    nc.sync.dma_start(
                        out=s32[gq * 4:(gq + 1) * 4, :],
                        in_=scrow[:, gq * 512:(gq + 1) * 512])
            else:
                nc.sync.dma_start(out=s32[:], in_=scrow[:])
            mx = wp.tile([GBL, 1], F32, name="mx")
            negmx = wp.tile([GBL, 1], F32, name="negmx")
            nc.vector.tensor_reduce(out=mx[:], in_=s32[:], axis=AX.X, op=ALU.max)
            nc.vector.tensor_scalar_mul(negmx[:], mx[:], -1.0)
            if FSUB >= 5:
                e32 = wp.tile([GBL, N], F32, name="e32")
                sume = wp.tile([GBL, 1], F32, name="sume")
                nc.scalar.activation(out=e32[:], in_=s32[:], func=AF.Exp,
                                     bias=negmx[:], accum_out=sume[:])
                lsum = wp.tile([GBL, 1], F32, name="lsum")
                nc.scalar.activation(out=lsum[:], in_=sume[:], func=AF.Ln)
            if FSUB >= 6:
                pick = wp.tile([GBL, 1], F32, name="pick")
                pscr = wp.tile([GBL, N], F32, name="pscr")
                nc.vector.tensor_mul(out=pscr[:], in0=s32[:], in1=sel_s)
                nc.vector.tensor_reduce(out=pick[:], in_=pscr[:], axis=AX.X,
                                        op=ALU.add)
                t3 = wp.tile([GBL, 1], F32, name="t3")
                nc.vector.tensor_add(out=t3[:], in0=mx[:], in1=lsum[:])
                nc.vector.tensor_sub(out=t3[:], in0=t3[:], in1=pick[:])
                nc.vector.tensor_add(out=colacc[:], in0=colacc[:], in1=t3[:])

            # ---------- gcn propagate: T layers ----------
            if PH < 4:
                continue
            for t in range(T):
                hVn = wp.tile([128, GBL, NF, 128], BF, name="hVn")
                for f in range(NF):
                    for gb in range(GBL // 4):
                        tps = pp.tile([128, 4, 128], BF, name="tps", tag="tp")
                        for j in range(4):
                            nc.tensor.transpose(
                                out=tps[:, j, :], in_=hVT[:, f, gb * 4 + j, :],
                                identity=identity[:])
                        eng = nc.vector if (gb % 2 == 0) else nc.scalar
                        if eng is nc.vector:
                            nc.vector.tensor_copy(
                                out=hVn[:, gb * 4:gb * 4 + 4, f, :], in_=tps[:])
                        else:
                            nc.scalar.copy(
                                out=hVn[:, gb * 4:gb * 4 + 4, f, :], in_=tps[:])
                m1T = wp.tile([128, NF, GBL, 128], BF, name="m1T")
                for gb in range(GBL // 4):
                    m1ps = pp.tile([128, 4, NF, 128], F32, name="m1ps", tag="mp", bufs=1)
                    for j in range(4):
                        g = gb * 4 + j
                        for f in range(NF):
                            nc.tensor.matmul(
                                out=m1ps[:, j, f, :], lhsT=hVn[:, g, f, :],
                                rhs=AT[:, g, :], start=True, stop=True)
                    # evacuate [128, 4, NF, 128] -> m1T[:, :, gb4, :] (note dim swap)
                    for f in range(NF):
                        eng_v = (gb % 2 == 0)
                        if eng_v:
                            nc.vector.tensor_copy(
                                out=m1T[:, f, gb * 4:gb * 4 + 4, :],
                                in_=m1ps[:, :, f, :])
                        else:
                            nc.scalar.copy(
                                out=m1T[:, f, gb * 4:gb * 4 + 4, :],
                                in_=m1ps[:, :, f, :])
                m1Tf = [m1T[:, f, :, :].rearrange("p g s -> p (g s)")
                        for f in range(NF)]
                for ko in range(NF):
                    for ch in range(NCH):
                        cols = slice(ch * 512, (ch + 1) * 512)
                        mm2ps = pp.tile([128, 512], F32, name="mm2ps", tag="zp")
                        for ki in range(NF):
                            nc.tensor.matmul(
                                out=mm2ps[:], lhsT=wgcn[:, t, ki, ko, :],
                                rhs=m1Tf[ki][:, cols],
                                start=(ki == 0), stop=(ki == NF - 1))
                        out_ap = hVT[:, ko, :, :].rearrange(
                            "p g s -> p (g s)")[:, cols]
                        nc.scalar.activation(
                            out=out_ap, in_=mm2ps[:], func=AF.Relu,
                            bias=bgcn[:, t, ko:ko + 1])

            # ---------- update graph repr ----------
            if PH >= 5:
                readout_full()

        # ---- finalize: loss = sum(rowacc) + sum(colacc), to DRAM ----
        r1 = cp.tile([1, 1], F32)
        nc.vector.tensor_reduce(out=r1[:], in_=rowacc[:], axis=AX.X, op=ALU.add)
        cps = pp.tile([1, 1], F32, name="cps", tag="sp")
        nc.tensor.matmul(out=cps[:], lhsT=colacc[:], rhs=ones32[:],
                         start=True, stop=True)
        losssb = cp.tile([1, 1], F32)
        nc.vector.tensor_add(out=losssb[:], in0=r1[:], in1=cps[:])
        nc.sync.dma_start(out=dout.ap(), in_=losssb[:])

    nc.compile()
    return nc


# --------------------------------------------------------------------------
# host-side input preparation
# --------------------------------------------------------------------------

def _bf(x):
    return np.ascontiguousarray(x).astype(ml_dtypes.bfloat16)


def _f32(x):
    return np.ascontiguousarray(x, dtype=np.float32)


def _tile_w(w, nki, nko):
    """[K, M] -> [128, nki, nko, 128] (lhsT tiles)."""
    K, M = w.shape
    assert K == nki * 128 and M == nko * 128
    return np.ascontiguousarray(
        w.reshape(nki, 128, nko, 128).transpose(1, 0, 2, 3).reshape(128, -1))


def _tile_b(b, n):
    return np.ascontiguousarray(b.reshape(n, 128).T)


def _prep_inputs(inputs):
    inp = {k: np.asarray(v) for k, v in inputs.items()}
    f32 = np.float32

    # adjacency blocks AT[g, s, d]
    src = inp["src"].astype(np.int64)
    dst = inp["dst"].astype(np.int64)
    flat = np.bincount(src * N + (dst % N), minlength=B * N * N)
    ATh = flat.reshape(B, N, N).astype(f32)

    # weights (shared across cores)
    shared = {
        "wfan1": _bf(_tile_w(inp["fan_w1"], 4, 4)),
        "bfan1": _f32(_tile_b(inp["fan_b1"], 4)),
        "wfan2d": _bf((inp["fan_w2"][:, 1] - inp["fan_w2"][:, 0]).reshape(4, 128).T),
        "wfinit1": _bf(_tile_w(inp["finit_w1"], 4, 4)),
        "bfinit1": _f32(_tile_b(inp["finit_b1"], 4)),
        "wfinit2": _bf(_tile_w(inp["finit_w2"], 4, 2)),
        "bfinit2": _f32(_tile_b(inp["finit_b2"], 2)),
        "wfae1": _bf(_tile_w(inp["fae_w1"], 6, 6)),
        "bfae1": _f32(_tile_b(inp["fae_b1"], 6)),
        "wfae2": _bf(inp["fae_w2"].reshape(6, 128).T),
        "wfs1a": _bf(_tile_w(inp["fs_w1"][:D], 2, 4)),
        "wfs1b": _bf(_tile_w(inp["fs_w1"][D:], 2, 4)),
        "bfs1": _f32(_tile_b(inp["fs_b1"], 4)),
        "wfs2": _bf(inp["fs_w2"][:, 0].reshape(4, 128).T),
        "wgp": _bf(_tile_w(inp["gpW"], 2, 4)),
        "bgpN": _f32(_tile_b(N * inp["gpb"], 4)),
        "wgcn": _bf(np.concatenate(
            [_tile_w(inp["gcn_W"][t], 2, 2) for t in range(T)], axis=1)),
        "bgcn": _f32(np.stack(
            [inp["gcn_b"][t].reshape(2, 128).T for t in range(T)], axis=1
        ).reshape(128, T * 2)),
        "ind": _bf((np.arange(32)[:, None] == (np.arange(GBL * N) // N)[None, :])),
        "consts": _f32(np.array(
            [[inp["fan_b2"][1] - inp["fan_b2"][0], inp["fae_b2"][0],
              EPS, 1.0 + EPS]])),
    }

    hV0 = inp["hV0"].astype(f32)
    labn = inp["labels_node"].astype(f32)   # [S, B]
    labe = inp["labels_edge"].astype(f32)
    sel = inp["node_select"]
    selhot = (np.arange(N)[None, None, :] == sel[:, :, None]).astype(f32)  # [S,B,N]

    in_maps = []
    for c in range(NCORES):
        gs = slice(c * GBL, (c + 1) * GBL)
        # hVT[p, f, g, s] = hV[(g*128+s), f*128+p]
        hvc = hV0.reshape(B, N, NF, 128)[gs]            # [GBL, s, f, p]
        hVT = np.ascontiguousarray(
            hvc.transpose(3, 2, 0, 1).reshape(128, -1))
        ATc = np.ascontiguousarray(
            ATh[gs].transpose(1, 0, 2).reshape(128, -1))  # [s(p), g, d]
        m = dict(shared)
        m["hVT0"] = _bf(hVT)
        m["AT"] = _bf(ATc)
        m["labn"] = _f32(labn[:, gs].reshape(1, S * GBL))
        m["labe"] = _f32(labe[:, gs].reshape(1, S * GBL))
        # selhot tile [32, S*N]: partition = graph-in-core
        m["selhot"] = _f32(
            selhot[:, gs].transpose(1, 0, 2).reshape(GBL, S * N))
        in_maps.append(m)
    return in_maps


# --------------------------------------------------------------------------
# public entry
# --------------------------------------------------------------------------

def kernel(**inputs) -> np.ndarray:
    global _BUILT
    if _BUILT is None:
        _BUILT = _build()
    nc = _BUILT
    in_maps = _prep_inputs(inputs)
    res = bass_utils.run_bass_kernel_spmd(
        nc, in_maps, core_ids=list(range(NCORES)))
    total = np.float32(0.0)
    for r in res.results:
        total += r["lossout"].reshape(())
    return np.float32(total / B)
